# revision 1
# baseline (speedup 1.0000x reference)
"""Black-oil PINO loss kernel for 8 Trainium2 NeuronCores.

Contract: kernel(**inputs) takes FULL f32 inputs [B=8,T=10,NZ=4,NX=128,NY=128]
and returns (p_loss, s_loss) as full f32 arrays, computed on 8 NeuronCores
(batch sharded, one batch element per core, no cross-core communication).

Math (constant-folded from the reference):
    prior    = shift_t(water_sat), prior[0] = siniuse = Swini[0,0,0,0,0]
    mw2      = Square(sigw*prior + betw)         # = 640*Mw
    mo2      = Square(sigo*prior + beto)         # = 640*Mo
    Dx/Dy    = raw central diff (f-b), DD = f-2c+b, edge-replicated
    dd       = DDx(p) + DDy(p)
    pd       = perm*dd
    U        = Dx(perm0)*Dx(p) + Dy(perm0)*Dy(p)
    sw       = cw*U + mw2*pd        # cw = 0.25*mw2(siniuse): t=0 prior is a
    so       = co*U + mo2*pd        # scalar, so grad(a1_0) = c * grad(perm0)
    p_loss   = cQ*Q + sw + so
    s_loss   = -(cQ*Qw + sw)
The saturation-accumulation term Phi*(dsw/dta)*dxf*1e-5 is <= 2.4e-10 while
|s_loss| ~ 2.7e3 (13 orders below f32 output noise), so it is dropped; Phi,
Time, Pini are then unused and never shipped to the device.

Device-side layout is [x(partitions), t, z, y(contiguous)], fp16. The host
pre-pads pressure/perm0 along y (edge replication) and folds the cQ scale
into the fp16 cast of Q/Qw. x stencils run on TensorE as 128x128
shift-matrix matmuls; y stencils are Id/-Id matmuls over y-shifted access
patterns of the padded pressure, accumulated into PSUM. Dx/Dy/dd land in
one 3-bank PSUM tile per timestep, moved to SBUF fp16 by a single ScalarE
copy. ScalarE also computes the Square mobilities (batched over all t);
VectorE runs the 13 remaining elementwise ops per 2-timestep block.
All inputs are loaded into resident SBUF tiles by a few large DMAs on the
sync (HWDGE) queue, ordered so block 0's dependencies land first; consts
are packed into a single tensor (f32 scalar columns bit-cast into it).
"""

import numpy as np

B, T, NZ, NX, NY = 8, 10, 4, 128, 128
N_CORES = 8
TB = 2            # timesteps per elementwise block
NBLK = T // TB
PW = NY + 4       # padded y width; data at [2:130], edge pads at 1 and 130

# folded constants
CQ = 5000.0 * 1e-5 / 128.0                 # dxf*1e-5*UIR
_S640 = np.sqrt(640.0)                     # 640 = dxf*1e-5*1000*128^2*500
_SO = np.sqrt(640.0 / 2.75)                # Mo carries 1/(UO*BO) = 1/2.75
SIGW, BETW = 1.25 * _S640, -0.125 * _S640
SIGO, BETO = -1.25 * _SO, 1.125 * _SO
GSCALE = 0.25                              # k1/k2 ratio: 160/640


def _shift_matrices():
    """lhsT (=M^T) matrices for out = M @ p along the partition (x) axis."""
    sx = np.zeros((NX, NX), np.float32)    # f - b, edge clamped
    for i in range(NX):
        f, b = min(i + 1, NX - 1), max(i - 1, 0)
        sx[i, f] += 1.0
        sx[i, b] -= 1.0
    sxx = np.zeros((NX, NX), np.float32)   # f - 2c + b, edge clamped
    for i in range(NX):
        f, b = min(i + 1, NX - 1), max(i - 1, 0)
        sxx[i, f] += 1.0
        sxx[i, b] += 1.0
        sxx[i, i] -= 2.0
    m1 = sxx - 2.0 * np.eye(NX, dtype=np.float32)  # folds the y-center -2c
    ident = np.eye(NX, dtype=np.float32)
    return (np.ascontiguousarray(sx.T), np.ascontiguousarray(m1.T),
            ident, np.ascontiguousarray(-ident))


_NC_CACHE = {}


def _build_nc():
    import sys
    if '/opt/trn_rl_repo' not in sys.path:
        sys.path.insert(0, '/opt/trn_rl_repo')
    import concourse.bacc as bacc
    import concourse.tile as tile
    import concourse.mybir as mybir

    if 'nc' in _NC_CACHE:
        return _NC_CACHE['nc']

    CDT = mybir.dt.float16
    F32 = mybir.dt.float32
    AO = mybir.AluOpType
    AF = mybir.ActivationFunctionType

    nc = bacc.Bacc("TRN2", target_bir_lowering=False, debug=False,
                   enable_asserts=False, num_devices=N_CORES)

    # wcat packs the 4 shift matrices + 6 f32 scalar columns (bit-cast to fp16)
    WCW = 4 * NX + 12
    wcat_in = nc.dram_tensor('wcat', [NX, WCW], CDT, kind="ExternalInput").ap()
    perm0p_in = nc.dram_tensor('perm0p', [NX, NZ, PW], CDT, kind="ExternalInput").ap()
    press = nc.dram_tensor('press', [NX, T, NZ, PW], CDT, kind="ExternalInput").ap()
    perm = nc.dram_tensor('perm', [NX, T, NZ, NY], CDT, kind="ExternalInput").ap()
    sat_in = nc.dram_tensor('sat', [NX, T - 1, NZ, NY], CDT, kind="ExternalInput").ap()
    qs_in = nc.dram_tensor('qs', [NX, T, NZ, NY], CDT, kind="ExternalInput").ap()
    qws_in = nc.dram_tensor('qws', [NX, T, NZ, NY], CDT, kind="ExternalInput").ap()
    out_ps = nc.dram_tensor('out_ps', [NX, 2, T, NZ, NY], CDT,
                            kind="ExternalOutput").ap()

    BLOCKS = [(0, 2), (2, 4), (6, 4)]  # (t0, nt) per elementwise block

    with tile.TileContext(nc) as tc:
        with (
            tc.tile_pool(name="consts", bufs=1) as cpool,
            tc.tile_pool(name="big", bufs=1) as bpool,
            tc.tile_pool(name="work", bufs=2) as wpool,
            tc.tile_pool(name="psum", bufs=2, space="PSUM") as ppool,
            tc.tile_pool(name="gsum", bufs=1, space="PSUM") as gppool,
        ):
            # ---- consts (one DMA) + earliest-critical input chunks ----
            wcat = cpool.tile([NX, WCW], CDT, tag='wcat')
            nc.sync.dma_start(wcat[:], wcat_in)
            press_all = bpool.tile([NX, T, NZ, PW], CDT, tag='press_all')
            b0 = BLOCKS[0][1]
            nc.sync.dma_start(press_all[:, :b0], press[:, :b0])
            perm0p = cpool.tile([NX, NZ, PW], CDT, tag='perm0p')
            nc.sync.dma_start(perm0p[:], perm0p_in)
            wsx, wm1, wid, wni = (wcat[:, k * NX:(k + 1) * NX] for k in range(4))
            ccat = wcat[:, 4 * NX:4 * NX + 12].bitcast(F32)
            mw0c, mo0c, cwc, coc, betwc, betoc = (ccat[:, k:k + 1] for k in range(6))

            # ---- grad(perm0) fields ----
            dpx = cpool.tile([NX, NZ, NY], CDT, tag='dpx')
            dpy = cpool.tile([NX, NZ, NY], CDT, tag='dpy')
            # (these copies run on the still-idle VectorE so ScalarE's queue
            # stays clear for block 0's mobility fills/Squares)
            dpx_ps = gppool.tile([NX, NZ, NY], F32, tag='gps')
            nc.tensor.matmul(dpx_ps[:], wsx, perm0p[:, :, 2:2 + NY],
                             start=True, stop=True)
            nc.vector.tensor_copy(dpx[:], dpx_ps[:])
            dpy_ps = gppool.tile([NX, NZ, NY], F32, tag='gps')
            nc.tensor.matmul(dpy_ps[:], wid, perm0p[:, :, 3:3 + NY],
                             start=True, stop=False)
            nc.tensor.matmul(dpy_ps[:], wni, perm0p[:, :, 1:1 + NY],
                             start=False, stop=True)
            nc.vector.tensor_copy(dpy[:], dpy_ps[:])

            # ---- remaining input loads ----
            perm_all = bpool.tile([NX, T, NZ, NY], CDT, tag='perm_all')
            sat_all = bpool.tile([NX, T - 1, NZ, NY], CDT, tag='sat_all')
            qs_all = bpool.tile([NX, T, NZ, NY], CDT, tag='qs_all')
            qws_all = bpool.tile([NX, T, NZ, NY], CDT, tag='qws_all')
            nc.sync.dma_start(perm_all[:, :b0], perm[:, :b0])
            nc.sync.dma_start(sat_all[:], sat_in)
            nc.sync.dma_start(qs_all[:, :b0], qs_in[:, :b0])
            nc.sync.dma_start(qws_all[:, :b0], qws_in[:, :b0])
            nc.sync.dma_start(press_all[:, b0:], press[:, b0:])
            nc.sync.dma_start(perm_all[:, b0:], perm[:, b0:])
            nc.sync.dma_start(qs_all[:, b0:], qs_in[:, b0:])
            nc.sync.dma_start(qws_all[:, b0:], qws_in[:, b0:])

            # ---- per-block pipeline ----
            for t0, nt in BLOCKS:
                tsl = slice(t0, t0 + nt)
                bdpx = dpx[:].unsqueeze(1).to_broadcast((NX, nt, NZ, NY))
                bdpy = dpy[:].unsqueeze(1).to_broadcast((NX, nt, NZ, NY))

                # mobilities for this block (prior = sat shifted by one t)
                mw2 = wpool.tile([NX, nt, NZ, NY], CDT, tag=f'mw2_{t0}', bufs=1,
                                 name=f'mw2_{t0}')
                mo2 = wpool.tile([NX, nt, NZ, NY], CDT, tag=f'mo2_{t0}', bufs=1,
                                 name=f'mo2_{t0}')
                if t0 == 0:
                    nc.scalar.activation(mw2[:, 0], perm0p[:, :, 2:2 + NY],
                                         AF.Identity, bias=mw0c, scale=0.0)
                    nc.scalar.activation(mo2[:, 0], perm0p[:, :, 2:2 + NY],
                                         AF.Identity, bias=mo0c, scale=0.0)
                    nc.scalar.activation(mw2[:, 1:nt], sat_all[:, 0:nt - 1],
                                         AF.Square, bias=betwc, scale=SIGW)
                    nc.scalar.activation(mo2[:, 1:nt], sat_all[:, 0:nt - 1],
                                         AF.Square, bias=betoc, scale=SIGO)
                else:
                    nc.scalar.activation(mw2[:], sat_all[:, t0 - 1:t0 - 1 + nt],
                                         AF.Square, bias=betwc, scale=SIGW)
                    nc.scalar.activation(mo2[:], sat_all[:, t0 - 1:t0 - 1 + nt],
                                         AF.Square, bias=betoc, scale=SIGO)

                stg = wpool.tile([NX, nt, 3, NZ, NY], CDT, tag=f'stg_{t0}', bufs=1,
                                 name=f'stg_{t0}')
                for i in range(nt):
                    t = t0 + i
                    center = press_all[:, t, :, 2:2 + NY]
                    minus = press_all[:, t, :, 1:1 + NY]
                    plus = press_all[:, t, :, 3:3 + NY]
                    ps = ppool.tile([NX, 3, NZ, NY], F32, tag='ps')
                    nc.tensor.matmul(ps[:, 0], wsx, center, start=True, stop=True)
                    nc.tensor.matmul(ps[:, 1], wid, plus, start=True, stop=False)
                    nc.tensor.matmul(ps[:, 1], wni, minus, start=False, stop=True)
                    nc.tensor.matmul(ps[:, 2], wm1, center, start=True, stop=False)
                    nc.tensor.matmul(ps[:, 2], wid, plus, start=False, stop=False)
                    nc.tensor.matmul(ps[:, 2], wid, minus, start=False, stop=True)
                    if t0 == 0:
                        # VectorE is idle during the fill; staging block 0 on
                        # it skips ScalarE's queued fills/Squares
                        nc.vector.tensor_copy(stg[:, i], ps[:])
                    else:
                        nc.scalar.copy(stg[:, i], ps[:])

                dxs = stg[:, :, 0]
                dys = stg[:, :, 1]
                dds = stg[:, :, 2]

                shp = [NX, nt, NZ, NY]
                pd = wpool.tile(shp, CDT, tag='pd', name='pd')
                ux = wpool.tile(shp, CDT, tag='ux', name='ux')
                uy = wpool.tile(shp, CDT, tag='uy', name='uy')
                uu = wpool.tile(shp, CDT, tag='uu', name='uu')
                nc.vector.tensor_mul(pd[:], perm_all[:, tsl], dds)
                nc.vector.tensor_mul(ux[:], bdpx, dxs)
                nc.vector.tensor_mul(uy[:], bdpy, dys)
                nc.vector.tensor_add(uu[:], ux[:], uy[:])
                mwd = wpool.tile(shp, CDT, tag='mwd', name='mwd')
                mod = wpool.tile(shp, CDT, tag='mod', name='mod')
                nc.vector.tensor_mul(mwd[:], mw2[:], pd[:])
                nc.vector.tensor_mul(mod[:], mo2[:], pd[:])
                cwu = wpool.tile(shp, CDT, tag='cwu', name='cwu')
                cou = wpool.tile(shp, CDT, tag='cou', name='cou')
                nc.vector.tensor_scalar(cwu[:], uu[:], cwc, None, op0=AO.mult)
                nc.vector.tensor_scalar(cou[:], uu[:], coc, None, op0=AO.mult)
                sw = wpool.tile(shp, CDT, tag='sw', name='sw')
                so = wpool.tile(shp, CDT, tag='so', name='so')
                nc.vector.tensor_add(sw[:], cwu[:], mwd[:])
                nc.vector.tensor_add(so[:], cou[:], mod[:])
                s_out = wpool.tile(shp, CDT, tag='s_out', name='s_out')
                nc.vector.tensor_sub(s_out[:], qws_all[:, tsl], sw[:])
                nc.sync.dma_start(out_ps[:, 0, tsl], s_out[:])
                p1 = wpool.tile(shp, CDT, tag='p1', name='p1')
                p_out = wpool.tile(shp, CDT, tag='p_out', name='p_out')
                nc.vector.tensor_add(p1[:], qs_all[:, tsl], so[:])
                nc.vector.tensor_add(p_out[:], p1[:], sw[:])
                nc.sync.dma_start(out_ps[:, 1, tsl], p_out[:])

    nc.compile()
    _NC_CACHE['nc'] = nc
    return nc


def kernel(pressure, perm, Q, Qw, Time, Pini, Phi, Swini, water_sat):
    import sys
    if '/opt/trn_rl_repo' not in sys.path:
        sys.path.insert(0, '/opt/trn_rl_repo')
    from concourse.bass_utils import run_bass_kernel_spmd

    nc = _build_nc()

    sini = float(np.asarray(Swini[0, 0, 0, 0, 0]))
    mw0 = np.float32((SIGW * sini + BETW) ** 2)
    mo0 = np.float32((SIGO * sini + BETO) ** 2)
    sxT, m1T, idm, nim = _shift_matrices()
    ccat = np.empty((NX, 6), np.float32)
    ccat[:, 0] = mw0
    ccat[:, 1] = mo0
    ccat[:, 2] = GSCALE * mw0
    ccat[:, 3] = GSCALE * mo0
    ccat[:, 4] = BETW
    ccat[:, 5] = BETO
    wcat = np.concatenate(
        [np.stack([sxT, m1T, idm, nim], axis=1).astype(np.float16)
         .reshape(NX, 4 * NX),
         ccat.view(np.float16)], axis=1)  # [NX, 4*NX+12]

    def to_xtzy(a, scale=None):  # [T,NZ,NX,NY] -> [NX,T,NZ,NY] fp16 contiguous
        a = np.asarray(a).transpose(2, 0, 1, 3)
        if scale is not None:
            a = a * scale
        return np.ascontiguousarray(a, dtype=np.float16)

    def pad_y(x):  # [NX, ..., NY] -> [NX, ..., NY+4] edge-padded fp16
        shp = x.shape[:-1] + (PW,)
        out = np.zeros(shp, np.float16)
        out[..., 2:2 + NY] = x
        out[..., 1] = x[..., 0]
        out[..., 2 + NY] = x[..., NY - 1]
        return out

    in_maps = []
    for c in range(N_CORES):
        perm_x = to_xtzy(perm[c])
        in_maps.append({
            'wcat': wcat,
            'press': pad_y(to_xtzy(pressure[c])),
            'perm': perm_x,
            'perm0p': pad_y(perm_x[:, 0]),
            'qs': to_xtzy(Q[c], CQ),
            'qws': to_xtzy(Qw[c], -CQ),
            'sat': to_xtzy(water_sat[c, :T - 1]),
            **{},
        })

    res = run_bass_kernel_spmd(nc, in_maps, core_ids=list(range(N_CORES)))

    p_loss = np.empty((B, T, NZ, NX, NY), np.float32)
    s_loss = np.empty((B, T, NZ, NX, NY), np.float32)
    for c in range(N_CORES):
        ps = res.results[c]['out_ps'].astype(np.float32)
        s_loss[c] = ps[:, 0].transpose(1, 2, 0, 3)
        p_loss[c] = ps[:, 1].transpose(1, 2, 0, 3)
    return p_loss, s_loss

